# revision 48
# baseline (speedup 1.0000x reference)
"""3-layer GAT (PyG-style GATConv x3 + global mean pool) on 8 trn2 NeuronCores.

Strategy: nodes are dealt round-robin to the 8 cores (dst-sharding), sorted
within each core by neighbor-bucket profile so slot grids need little padding.
All 3 layers run in ONE SPMD program (hT stays in SBUF between layers).  Per
layer: a dense phase (hW = h @ W + the per-node attention logit halves)
publishes 256B table rows [fp8 h | f16 al_s | pad] (f16 for layer 3) which
are AllGathered across cores.  Edge phase: the table is split into 4
row-buckets of <=32k rows (dma_gather idx are int16, replicated over the 8
Q7 groups); per iteration (nb chunks of 128 dst nodes, per-bucket padded slot
counts Kb) dma_gathers of <=1024 rows each (SWDGE ring limit) pull the slot
rows, then e = exp(leakyrelu(al_s + al_d)) and alpha = e / sum_k e are formed
(exp + alpha-over-channel expansion on ACT) and the weighted neighbor sum
sum_k alpha*h is accumulated on the PE as transposes into PSUM - landing
already in the transposed [ch, node] layout the next layer's dense phase
needs; ReLU+copy to SBUF happens on ACT.  Layer 3 (1 head, 32 ch, no concat)
keeps a DVE fold-tree and a PE column-sum; the host divides by N and adds b3.
All per-core programs are identical (SPMD); per-core data differs.
"""
import numpy as np
import concourse.bass as bass
import concourse.bacc as bacc
import concourse.mybir as mybir
import concourse.tile as tile
from concourse.masks import make_identity

P = 128
NB = 4                 # gather-table buckets (dma_gather idx is int16)
NEG_SLOPE = 0.2
PAD_ALS = -30000.0     # al_s for padding rows: exp(lrelu(.)) == 0 in fp16
F32 = mybir.dt.float32
F16 = mybir.dt.float16
I32 = mybir.dt.int32
I16 = mybir.dt.int16
U8 = mybir.dt.uint8
FP8 = mybir.dt.float8e4
BATCH_AREA = 96        # max padded slots per edge-phase iteration
FP8_TBL = True         # fp8 h in the gather table (halves AllGather bytes)
NBCAP = 12             # max chunks per iteration (PSUM budget)


class Plan:
    pass


def make_plan(edge_index, N, ncores=8):
    E = edge_index.shape[1]
    src = np.concatenate([edge_index[0].astype(np.int64), np.arange(N, dtype=np.int64)])
    dst = np.concatenate([edge_index[1].astype(np.int64), np.arange(N, dtype=np.int64)])
    deg = np.bincount(dst, minlength=N)
    order0 = np.argsort(-deg, kind="stable")

    npc = (N + ncores - 1) // ncores
    n_chunks = (npc + P - 1) // P + 1  # last chunk is all-pad
    S = n_chunks * P
    BSZ = 2 * S  # bucket = 2 consecutive cores' rows

    # provisional core deal fixes each node's core (hence bucket = core//2)
    core_of = np.zeros(N, np.int64)
    for c in range(ncores):
        g = np.arange(npc) * ncores + c
        g = g[g < N]
        core_of[order0[g]] = c

    # CSR over dst
    eo = np.argsort(dst, kind="stable")
    src_sorted = src[eo]
    starts = np.zeros(N + 1, np.int64)
    np.cumsum(deg, out=starts[1:])

    # per-node bucket profile of its neighbor multiset
    src_b = core_of[src_sorted] // 2
    prof = np.zeros((N, NB), np.int32)
    for b in range(NB):
        m = (src_b == b).astype(np.int32)
        cs = np.concatenate([[0], np.cumsum(m)])
        prof[:, b] = cs[starts[1:]] - cs[starts[:-1]]

    # resort WITHIN each core by profile so chunks have near-uniform Kb
    # (core assignment must stay fixed: it defines each node's bucket)
    node_at = np.full((ncores, S), -1, np.int64)
    for c in range(ncores):
        g = np.arange(npc) * ncores + c
        g = g[g < N]
        nodes = order0[g]
        key = np.lexsort((-prof[nodes, 1], -prof[nodes, 0],
                          -prof[nodes].max(axis=1)))
        nodes = nodes[key]
        node_at[c, :len(nodes)] = nodes
    row_of = np.zeros(N, np.int64)
    for c in range(ncores):
        m = node_at[c] >= 0
        row_of[node_at[c][m]] = c * S + np.nonzero(m)[0]

    maskD = np.zeros((ncores, P, n_chunks), np.float32)
    for c in range(ncores):
        for j in range(n_chunks):
            for p in range(P):
                if node_at[c, j * P + p] < 0:
                    maskD[c, p, j] = 1.0

    # per-chunk cross-core bucket maxima (SPMD: one grid for all cores)
    Kb = np.zeros((n_chunks, NB), np.int64)
    for j in range(n_chunks):
        stratum = node_at[:, j * P:(j + 1) * P].ravel()
        stratum = stratum[stratum >= 0]
        if len(stratum):
            Kb[j] = prof[stratum].max(axis=0)
    if Kb[n_chunks - 1].sum() == 0:
        Kb[n_chunks - 1, 0] = 2  # all-pad chunk: force 2 pad slots -> psum = 0

    # greedy area-bounded iteration grouping
    iters = []
    j = 0
    while j < n_chunks:
        cur = Kb[j].copy()
        nb = 1
        while j + nb < n_chunks and nb < NBCAP:
            nk = np.maximum(cur, Kb[j + nb])
            if (nb + 1) * nk.sum() > BATCH_AREA:
                break
            cur = nk
            nb += 1
        iters.append((tuple(int(x) for x in cur), j, nb))
        j += nb

    # per-core int16 idx streams: per (iter, bucket): nidx = 128*nb*Kb items,
    # item i -> (p = i%128, col = i//128), col = q*Kb + k; striped into 16
    # partitions (idx16[i%16, i//16]) and replicated to all 8 Q7 groups.
    pad_local = S - 1  # core 2b's all-pad last row, local to bucket b
    neigh = {}  # node -> list of NB arrays of local rows
    idx_parts = [[] for _ in range(ncores)]
    for c in range(ncores):
        for (Kbt, c0, nb) in iters:
            for b in range(NB):
                kb = Kbt[b]
                if kb == 0:
                    continue
                nidx = 128 * nb * kb
                lin = np.full(nidx, pad_local, np.int64)
                for q in range(nb):
                    nodes = node_at[c, (c0 + q) * P:(c0 + q + 1) * P]
                    for p in range(P):
                        n = nodes[p]
                        if n < 0:
                            continue
                        nb_rows = neigh.get(n)
                        if nb_rows is None:
                            rows = row_of[src_sorted[starts[n]:starts[n + 1]]]
                            bb = rows // BSZ
                            nb_rows = [rows[bb == x] - x * BSZ for x in range(NB)]
                            neigh[n] = nb_rows
                        r = nb_rows[b]
                        cols = q * kb + np.arange(len(r))
                        lin[cols * 128 + p] = r
                assert lin.max() < 32768
                idx16 = lin.reshape(-1, 16).T.astype(np.int16)  # [16, nidx/16]
                idx_parts[c].append(np.tile(idx16, (8, 1)))
    idx = np.stack([np.concatenate(parts, axis=1) for parts in idx_parts])

    pl = Plan()
    pl.N, pl.E, pl.ncores = N, E, ncores
    pl.npc, pl.n_chunks, pl.S, pl.BSZ = npc, n_chunks, S, BSZ
    pl.Kb, pl.iters, pl.node_at, pl.row_of = Kb, iters, node_at, row_of
    pl.idx, pl.maskD = idx.astype(np.int16), maskD
    pl.idxtot = idx.shape[2]
    return pl


def layer_inputs(pl, layer, hins, W, a_src, a_dst, b):
    """hins: list of per-core [C0,S] arrays (xT fp32 for layer 0, hT fp16 else)."""
    av = np.stack([np.asarray(a_src).reshape(-1), np.asarray(a_dst).reshape(-1)])
    av = np.tile(av[:, None, :], (1, P, 1)).reshape(2 * P, -1).astype(np.float32)
    Wd = np.asarray(W, np.float32 if layer == 0 else np.float16)
    ins = []
    for c in range(pl.ncores):
        d = {"hin": hins[c], "idx": pl.idx[c], "maskD": pl.maskD[c],
             f"W{layer}": Wd, f"av{layer}": av}
        if layer < 2:
            d[f"bv{layer}"] = np.tile(np.asarray(b, np.float32)[None, :], (P, 1))
        ins.append(d)
    return ins


def x_slices(pl, x):
    out = []
    for c in range(pl.ncores):
        xs = np.zeros((pl.S, x.shape[1]), np.float32)
        m = pl.node_at[c] >= 0
        xs[m] = x[pl.node_at[c][m]]
        out.append(np.ascontiguousarray(xs.T))
    return out


def build_program(pl, layer, C0=128, H=(8, 8, 1), CH=(16, 16, 32), ncores=8):
    OC = [H[i] * CH[i] for i in range(3)]
    RL = [136, 136, 34]          # compact hcat row (hW | al_s), f16 elems
    # gather table: 256B-multiple rows (dma_gather elem constraint).
    # FP8_TBL: L0/L1 uint8 rows [128 fp8 h | 8 f16 al_s | pad] (256B);
    # else f16 rows [128 h | 8 al_s | pad] (512B).  L2: f16 [32 h | al_s | pad].
    if FP8_TBL:
        REL = [256, 256, 128]    # table row in table-dtype elems
        TDT = [U8, U8, F16]
    else:
        REL = [256, 256, 128]
        TDT = [F16, F16, F16]
    S, n_chunks = pl.S, pl.n_chunks
    NC = ncores
    MAXB = max(nb for (_, _, nb) in pl.iters)
    L = layer
    fused = layer is None
    LAYERS = (0, 1, 2) if fused else (L,)

    nc = bacc.Bacc("TRN2", target_bir_lowering=False, debug=False, num_devices=NC)
    t_hin = nc.dram_tensor("hin", [C0, S],
                           F32 if (fused or L == 0) else F16,
                           kind="ExternalInput")
    t_idx = nc.dram_tensor("idx", [P, pl.idxtot], I16, kind="ExternalInput")
    t_maskD = nc.dram_tensor("maskD", [P, n_chunks], F32, kind="ExternalInput")
    t_W = {l: nc.dram_tensor(f"W{l}", [C0 if l == 0 else OC[0], OC[l]],
                             F32 if l == 0 else F16, kind="ExternalInput")
           for l in LAYERS}
    t_av = {l: nc.dram_tensor(f"av{l}", [2 * P, OC[l]], F32,
                              kind="ExternalInput") for l in LAYERS}
    t_bv = {l: nc.dram_tensor(f"bv{l}", [P, OC[l]], F32, kind="ExternalInput")
            for l in LAYERS if l < 2}
    if not fused and L < 2:
        t_hout = nc.dram_tensor("hout", [P, S], F16, kind="ExternalOutput")
    if fused or L == 2:
        t_y = nc.dram_tensor("y", [1, OC[2]], F32, kind="ExternalOutput")

    with tile.TileContext(nc) as tc:
        from contextlib import ExitStack
        with tc.tile_pool(name="res", bufs=1) as res, \
             tc.tile_pool(name="dram", bufs=1, space="DRAM") as dram, \
             tc.tile_pool(name="dn", bufs=2) as dn, \
             tc.tile_pool(name="dnp", bufs=2, space="PSUM") as dnp:
            hT = res.tile([P, S], F16)
            if not fused and L > 0:
                nc.sync.dma_start(hT[:], t_hin.ap())
            alD = res.tile([P, n_chunks * 8], F16)
            mask_sb = res.tile([P, n_chunks], F32)
            nc.sync.dma_start(mask_sb[:], t_maskD.ap())
            ident = res.tile([P, P], F16)
            make_identity(nc, ident[:])
            ones_col = res.tile([P, 1], F16)
            nc.gpsimd.memset(ones_col[:], 1.0)
            W_sb, av_sb, bv_sb = {}, {}, {}
            for l in LAYERS:
                w_t = res.tile(list(t_W[l].shape), F32 if l == 0 else F16,
                               tag=f"W{l}")
                nc.sync.dma_start(w_t[:], t_W[l].ap())
                W_sb[l] = w_t
                a_s = res.tile([P, OC[l]], F32, tag=f"as{l}")
                nc.sync.dma_start(a_s[:], t_av[l].ap()[0:P, :])
                a_d = res.tile([P, OC[l]], F32, tag=f"ad{l}")
                nc.sync.dma_start(a_d[:], t_av[l].ap()[P:2 * P, :])
                av_sb[l] = (a_s, a_d)
                if l < 2:
                    bv_t = res.tile([P, OC[l]], F32, tag=f"bv{l}")
                    nc.sync.dma_start(bv_t[:], t_bv[l].ap())
                    bv_sb[l] = bv_t

            hcat_loc = {l: dram.tile([S, REL[l]], TDT[l], name=f"hcl{l}")
                        for l in LAYERS}
            hcat_full = {l: dram.tile([NC * S, REL[l]], TDT[l], name=f"hcf{l}",
                                      addr_space="Shared") for l in LAYERS}

            for l in LAYERS:
                oc, heads, ch, rl, rel = OC[l], H[l], CH[l], RL[l], REL[l]
                # ---------------- dense ----------------
                DB = 4
                if True:
                    for it0 in range(0, n_chunks, DB):
                        nb = min(DB, n_chunks - it0)
                        if l == 0:
                            xin = dn.tile([C0, DB * P], F32, tag="xin")
                            nc.sync.dma_start(xin[:, :nb * P],
                                              t_hin.ap()[:, it0 * P:(it0 + nb) * P])
                        ps = dnp.tile([P, DB * oc], F32, tag="ps")
                        for q in range(nb):
                            lhsT = (xin[:, q * P:(q + 1) * P] if l == 0 else
                                    hT[:, (it0 + q) * P:(it0 + q + 1) * P])
                            nc.tensor.matmul(ps[:, q * oc:(q + 1) * oc], lhsT=lhsT,
                                             rhs=W_sb[l][:], start=True, stop=True)
                        psv = ps[:, :nb * oc].rearrange("p (q o) -> p q o", o=oc)
                        als_red = None
                        for which in range(2):
                            a_bc = av_sb[l][which][:].unsqueeze(1) \
                                .to_broadcast([P, nb, oc])
                            tmp = dn.tile([P, DB * oc], F32, tag=f"tmp{which}")
                            nc.vector.tensor_tensor(
                                out=tmp[:, :nb * oc].rearrange("p (q o) -> p q o", o=oc),
                                in0=psv, in1=a_bc, op=mybir.AluOpType.mult)
                            red = dn.tile([P, DB * 8], F32, tag=f"red{which}")
                            nc.vector.tensor_reduce(
                                out=red[:, :nb * heads],
                                in_=tmp[:, :nb * oc].rearrange(
                                    "p (q h c) -> p q h c", h=heads, c=ch),
                                axis=mybir.AxisListType.X, op=mybir.AluOpType.add)
                            if which == 0:
                                als_red = red
                            else:
                                dv = alD[:, it0 * 8:(it0 + nb) * 8] \
                                    .rearrange("p (q e) -> p q e", e=8)[:, :, :heads]
                                nc.vector.tensor_copy(
                                    out=dv, in_=red[:, :nb * heads]
                                    .rearrange("p (q h) -> p q h", h=heads))
                        if l < 2 and FP8_TBL:
                            hc = dn.tile([P, DB * rl], F16, tag=f"hc{l}")
                            hcv = hc[:, :nb * rl].rearrange("p (q r) -> p q r", r=rl)
                            b_bc = bv_sb[l][:].unsqueeze(1).to_broadcast([P, nb, oc])
                            nc.vector.tensor_tensor(out=hcv[:, :, 0:oc], in0=psv,
                                                    in1=b_bc, op=mybir.AluOpType.add)
                            nc.vector.tensor_copy(
                                out=hcv[:, :, oc:oc + heads],
                                in_=als_red[:, :nb * heads]
                                .rearrange("p (q h) -> p q h", h=heads))
                            # pack table row: fp8 h (ACT converts) | f16 al_s
                            hc8 = dn.tile([P, DB * rel], U8, tag="hc8")
                            hc8v = hc8[:, :nb * rel].rearrange("p (q e) -> p q e", e=rel)
                            nc.scalar.activation(
                                out=hc8v[:, :, 0:oc].bitcast(FP8),
                                in_=hcv[:, :, 0:oc],
                                func=mybir.ActivationFunctionType.Copy)
                            nc.vector.tensor_copy(
                                out=hc8v[:, :, oc:oc + 2 * heads].bitcast(F16),
                                in_=hcv[:, :, oc:oc + heads])
                            if it0 + nb == n_chunks:
                                nc.gpsimd.memset(
                                    hc8[:, (nb - 1) * rel + oc:
                                        (nb - 1) * rel + oc + 2 * heads]
                                    .bitcast(F16), PAD_ALS)
                            nc.sync.dma_start(
                                hcat_loc[l][:][it0 * P:(it0 + nb) * P, :]
                                .rearrange("(q p) r -> p q r", p=P), hc8v)
                        else:
                            hc = dn.tile([P, DB * rel], F16, tag=f"hcx{l}")
                            hcv = hc[:, :nb * rel].rearrange("p (q r) -> p q r", r=rel)
                            if l < 2:
                                b_bc = bv_sb[l][:].unsqueeze(1) \
                                    .to_broadcast([P, nb, oc])
                                nc.vector.tensor_tensor(
                                    out=hcv[:, :, 0:oc], in0=psv, in1=b_bc,
                                    op=mybir.AluOpType.add)
                            else:
                                nc.vector.tensor_copy(out=hcv[:, :, 0:oc], in_=psv)
                            nc.vector.tensor_copy(
                                out=hcv[:, :, oc:oc + heads],
                                in_=als_red[:, :nb * heads]
                                .rearrange("p (q h) -> p q h", h=heads))
                            if it0 + nb == n_chunks:
                                nc.gpsimd.memset(
                                    hc[:, (nb - 1) * rel + oc:
                                       (nb - 1) * rel + oc + heads], PAD_ALS)
                            nc.sync.dma_start(
                                hcat_loc[l][:][it0 * P:(it0 + nb) * P, :]
                                .rearrange("(q p) r -> p q r", p=P), hcv)
                # ---------------- allgather ----------------
                nc.gpsimd.collective_compute(
                    "AllGather", mybir.AluOpType.bypass,
                    replica_groups=[list(range(NC))],
                    ins=[hcat_loc[l].opt()], outs=[hcat_full[l].opt()])
                # ---------------- edge phase ----------------
                _es = ExitStack()
                eg = _es.enter_context(tc.tile_pool(name=f"eg{l}", bufs=2))
                eg1 = _es.enter_context(tc.tile_pool(name=f"eg1{l}", bufs=1))
                egp = _es.enter_context(
                    tc.tile_pool(name=f"egp{l}", bufs=2, space="PSUM"))
                relu = l < 2
                if True:
                    if l == 2:
                        y_acc = None
                        it_i = 0
                    it_mp = 0
                    idx_off = 0
                    nidx_regs = {}
                    for (Kbt, c0, nb) in pl.iters:
                        kbsum = sum(Kbt)
                        nstot = nb * kbsum
                        regions = []  # (bucket, col0, Kb)
                        col = 0
                        for b in range(NB):
                            if Kbt[b]:
                                regions.append((b, col, Kbt[b]))
                                col += nb * Kbt[b]
                        icols = 8 * nstot
                        idx_sb = eg.tile([P, 8 * BATCH_AREA], I16, tag="idx")
                        nc.sync.dma_start(idx_sb[:, :icols],
                                          t_idx.ap()[:, idx_off:idx_off + icols])
                        g = eg.tile([P, BATCH_AREA * rel], TDT[l], tag=f"g{l}")
                        ic = 0
                        for (b, col0, kb) in regions:
                            # SWDGE ring holds 1024 descriptors: cap each
                            # dma_gather at 8 slot-columns (1024 idxs).
                            ncols = nb * kb
                            done = 0
                            while done < ncols:
                                take = min(8, ncols - done)
                                nidx = 128 * take
                                c0_ = col0 + done
                                if nidx not in nidx_regs:
                                    nidx_regs[nidx] = nc.gpsimd.to_reg(nidx)
                                nc.gpsimd.dma_gather(
                                    out_ap=g[:, c0_ * rel:(c0_ + take) * rel]
                                    .rearrange("p (c e) -> p c e", e=rel),
                                    in_ap=hcat_full[l][:][b * pl.BSZ:
                                                          (b + 1) * pl.BSZ, :],
                                    idxs_ap=idx_sb[:, ic:ic + take * 8],
                                    num_idxs=nidx, num_idxs_reg=nidx_regs[nidx],
                                    elem_size=rel)
                                ic += take * 8
                                done += take
                        idx_off += icols
                        gv = g[:, :nstot * rel].rearrange("p (s e) -> p s e", e=rel)
                        if l < 2 and FP8_TBL:
                            gh = gv[:, :, 0:oc].bitcast(FP8)
                            gals = gv[:, :, oc:oc + 2 * heads].bitcast(F16)
                        else:
                            gh = gv[:, :, 0:oc]
                            gals = gv[:, :, oc:oc + heads]
                        # logits = al_s + al_d per region (al_d broadcast over k)
                        lg = eg1.tile([P, BATCH_AREA * 8], F16, tag="lg")
                        for (b, col0, kb) in regions:
                            al_d_bc = alD[:, c0 * 8:(c0 + nb) * 8] \
                                .rearrange("p (q e) -> p q e", e=8)[:, :, :heads] \
                                .unsqueeze(2).to_broadcast([P, nb, kb, heads])
                            nc.vector.tensor_tensor(
                                out=lg[:, col0 * heads:(col0 + nb * kb) * heads]
                                .rearrange("p (q k h) -> p q k h", k=kb, h=heads),
                                in0=gals[:, col0:col0 + nb * kb, :]
                                .rearrange("p (q k) h -> p q k h", k=kb),
                                in1=al_d_bc, op=mybir.AluOpType.add)
                        lg3 = eg1.tile([P, BATCH_AREA * 8], F16, tag="lg3")
                        nc.vector.scalar_tensor_tensor(
                            out=lg3[:, :nstot * heads], in0=lg[:, :nstot * heads],
                            scalar=NEG_SLOPE, in1=lg[:, :nstot * heads],
                            op0=mybir.AluOpType.mult, op1=mybir.AluOpType.max)
                        es = eg1.tile([P, BATCH_AREA * 8], F16, tag="es")
                        nc.scalar.activation(out=es[:, :nstot * heads],
                                             in_=lg3[:, :nstot * heads],
                                             func=mybir.ActivationFunctionType.Exp)
                        # denominator: per-region reduce over k, then sum regions
                        nreg = len(regions)
                        den4 = eg1.tile([P, NB * NBCAP * 8], F32, tag="den4")
                        for ri, (b, col0, kb) in enumerate(regions):
                            nc.vector.tensor_reduce(
                                out=den4[:, ri * nb * heads:(ri + 1) * nb * heads],
                                in_=es[:, col0 * heads:(col0 + nb * kb) * heads]
                                .rearrange("p (q k h) -> p q h k", k=kb, h=heads),
                                axis=mybir.AxisListType.X, op=mybir.AluOpType.add)
                        den = eg1.tile([P, NBCAP * 8], F32, tag="den")
                        if nreg > 1:
                            nc.vector.tensor_reduce(
                                out=den[:, :nb * heads],
                                in_=den4[:, :nreg * nb * heads]
                                .rearrange("p (r x) -> p x r", r=nreg),
                                axis=mybir.AxisListType.X, op=mybir.AluOpType.add)
                            dsum = den
                        else:
                            dsum = den4
                        den2 = eg1.tile([P, NBCAP * 8], F32, tag="den2")
                        m_bc = mask_sb[:, c0:c0 + nb].unsqueeze(2) \
                            .to_broadcast([P, nb, heads])
                        nc.vector.tensor_tensor(
                            out=den2[:, :nb * heads].rearrange("p (q h) -> p q h", h=heads),
                            in0=dsum[:, :nb * heads].rearrange("p (q h) -> p q h", h=heads),
                            in1=m_bc, op=mybir.AluOpType.add)
                        inv = eg1.tile([P, NBCAP * 8], F32, tag="inv")
                        nc.vector.reciprocal(out=inv[:, :nb * heads], in_=den2[:, :nb * heads])
                        # alpha = es / den (per-region: inv broadcast over k)
                        alpha = eg1.tile([P, BATCH_AREA * 8], F16, tag="alpha")
                        for (b, col0, kb) in regions:
                            inv_bc = inv[:, :nb * heads] \
                                .rearrange("p (q h) -> p q h", h=heads) \
                                .unsqueeze(2).to_broadcast([P, nb, kb, heads])
                            nc.vector.tensor_tensor(
                                out=alpha[:, col0 * heads:(col0 + nb * kb) * heads]
                                .rearrange("p (q k h) -> p q k h", k=kb, h=heads),
                                in0=es[:, col0 * heads:(col0 + nb * kb) * heads]
                                .rearrange("p (q k h) -> p q k h", k=kb, h=heads),
                                in1=inv_bc, op=mybir.AluOpType.mult)
                        if relu:
                            # alpha expanded over ch on ACT; weight on DVE (2x);
                            # PE accumulates transposes: psum = [ch, node].
                            al16 = eg1.tile([P, BATCH_AREA * 128], F16, tag="al16")
                            nc.scalar.activation(
                                out=al16[:, :nstot * oc].rearrange(
                                    "p (s h c) -> p s h c", h=heads, c=ch),
                                in_=alpha[:, :nstot * heads].rearrange(
                                    "p (s h) -> p s h", h=heads)
                                .unsqueeze(3).to_broadcast([P, nstot, heads, ch]),
                                func=mybir.ActivationFunctionType.Copy)
                            mp = eg1.tile([P, BATCH_AREA * 128], F16, tag="mp")
                            if FP8_TBL and it_mp % 2 == 0:
                                # rebalance: ACT converts fp8 h -> f16 so the
                                # multiply runs in DVE 2x mode for 1 in 2 iters
                                g16 = eg1.tile([P, BATCH_AREA * 128], F16, tag="g16")
                                nc.scalar.activation(
                                    out=g16[:, :nstot * oc].rearrange(
                                        "p (s c) -> p s c", c=oc),
                                    in_=gh,
                                    func=mybir.ActivationFunctionType.Copy)
                                mp_in0 = g16[:, :nstot * oc].rearrange(
                                    "p (s c) -> p s c", c=oc)
                            else:
                                mp_in0 = gh
                            it_mp += 1
                            nc.vector.tensor_tensor(
                                out=mp[:, :nstot * oc].rearrange("p (s c) -> p s c", c=oc),
                                in0=mp_in0,
                                in1=al16[:, :nstot * oc].rearrange("p (s c) -> p s c", c=oc),
                                op=mybir.AluOpType.mult)
                            ps = egp.tile([P, NBCAP * P], F32, tag="eps")
                            for q in range(nb):
                                mm = 0
                                for (b, col0, kb) in regions:
                                    for k in range(kb):
                                        s = col0 + q * kb + k
                                        nc.tensor.matmul(
                                            ps[:, q * P:(q + 1) * P],
                                            lhsT=mp[:, s * oc:(s + 1) * oc],
                                            rhs=ident[:], start=(mm == 0),
                                            stop=(mm == kbsum - 1))
                                        mm += 1
                            for q in range(nb):
                                nc.scalar.activation(
                                    out=hT[:, (c0 + q) * P:(c0 + q + 1) * P],
                                    in_=ps[:, q * P:(q + 1) * P],
                                    func=mybir.ActivationFunctionType.Relu)
                        else:
                            al_bc = alpha[:, :nstot * heads] \
                                .rearrange("p (s h) -> p s h", h=heads) \
                                .unsqueeze(3).to_broadcast([P, nstot, heads, ch])
                            mp = eg1.tile([P, BATCH_AREA * 32], F16, tag="mp3")
                            nc.vector.tensor_tensor(
                                out=mp[:, :nstot * oc].rearrange(
                                    "p (s h c) -> p s h c", h=heads, c=ch),
                                in0=gh.rearrange("p s (h c) -> p s h c", h=heads),
                                in1=al_bc, op=mybir.AluOpType.mult)
                            finals = []
                            for ri, (b, col0, kb) in enumerate(regions):
                                scrA = eg1.tile([P, BATCH_AREA * 16], F16, tag=f"sA{ri}")
                                scrB = eg1.tile([P, BATCH_AREA * 12], F16, tag=f"sB{ri}")
                                cur, curk, curoff = mp, kb, col0
                                while curk > 1:
                                    a_in = cur[:, curoff * oc:(curoff + nb * curk) * oc] \
                                        .rearrange("p (q k c) -> p q k c", k=curk, c=oc)
                                    if curk % 2 == 1:
                                        half = (curk + 1) // 2
                                        pair = curk - half
                                    else:
                                        half, pair = curk // 2, curk // 2
                                    dst_t = scrA if cur is not scrA else scrB
                                    o_v = dst_t[:, :nb * half * oc].rearrange(
                                        "p (q k c) -> p q k c", k=half, c=oc)
                                    nc.vector.tensor_tensor(
                                        out=o_v[:, :, 0:pair], in0=a_in[:, :, 0:pair],
                                        in1=a_in[:, :, half:half + pair],
                                        op=mybir.AluOpType.add)
                                    if half > pair:
                                        nc.vector.tensor_copy(out=o_v[:, :, pair:half],
                                                              in_=a_in[:, :, pair:half])
                                    cur, curk, curoff = dst_t, half, 0
                                finals.append((cur, curoff))
                            msum, moff = finals[0]
                            for ai, (f, foff) in enumerate(finals[1:]):
                                acc = eg1.tile([P, NBCAP * 32], F16, tag=f"acc{ai}")
                                nc.vector.tensor_tensor(
                                    out=acc[:, :nb * oc],
                                    in0=msum[:, moff * oc:(moff + nb) * oc],
                                    in1=f[:, foff * oc:(foff + nb) * oc],
                                    op=mybir.AluOpType.add)
                                msum, moff = acc, 0
                            yps = egp.tile([1, OC[2]], F32, tag="ysum")
                            for q in range(nb):
                                nc.tensor.matmul(
                                    yps[:], lhsT=ones_col[:],
                                    rhs=msum[:, (moff + q) * oc:(moff + q + 1) * oc],
                                    start=(q == 0), stop=(q == nb - 1))
                            nacc = res.tile([1, OC[2]], F32, tag=f"yacc{it_i % 2}")
                            if y_acc is None:
                                nc.vector.tensor_copy(out=nacc[:], in_=yps[:])
                            else:
                                nc.vector.tensor_tensor(out=nacc[:], in0=y_acc[:],
                                                        in1=yps[:],
                                                        op=mybir.AluOpType.add)
                            y_acc = nacc
                            it_i += 1
                _es.close()
            if fused or L == 2:
                nc.sync.dma_start(t_y.ap(), y_acc[:])
            else:
                nc.sync.dma_start(t_hout.ap(), hT[:])
    nc.compile()
    return nc


# ----------------------------------------------------------------- entry point

N_NODES, N_EDGES = 100000, 1600000
import os as _os
FUSED = _os.environ.get("GAT_FUSED", "1") == "1"
_CACHE = {}


def _get_compiled(edge_index):
    key = hash(edge_index.tobytes())
    if key not in _CACHE:
        pl = make_plan(edge_index, N_NODES, ncores=8)
        if FUSED:
            ncs = [build_program(pl, layer=None, C0=128, H=(8, 8, 1),
                                 CH=(16, 16, 32), ncores=8)]
        else:
            ncs = [build_program(pl, layer=l, C0=128, H=(8, 8, 1),
                                 CH=(16, 16, 32), ncores=8) for l in range(3)]
        _CACHE.clear()
        _CACHE[key] = (pl, ncs)
    return _CACHE[key]


def fused_inputs(pl, x, layer_params):
    hins = x_slices(pl, x)
    ins = []
    for c in range(pl.ncores):
        d = {"hin": hins[c], "idx": pl.idx[c], "maskD": pl.maskD[c]}
        for l, (W, a_s, a_d, b) in enumerate(layer_params):
            av = np.stack([np.asarray(a_s).reshape(-1),
                           np.asarray(a_d).reshape(-1)])
            av = np.tile(av[:, None, :], (1, P, 1)).reshape(2 * P, -1) \
                .astype(np.float32)
            d[f"W{l}"] = np.asarray(W, np.float32 if l == 0 else np.float16)
            d[f"av{l}"] = av
            if l < 2:
                d[f"bv{l}"] = np.tile(np.asarray(b, np.float32)[None, :],
                                      (P, 1))
        ins.append(d)
    return ins


def kernel(x, edge_index, W1, a_src1, a_dst1, b1, W2, a_src2, a_dst2, b2,
           W3, a_src3, a_dst3, b3):
    from concourse import bass_utils
    x = np.asarray(x, np.float32)
    edge_index = np.asarray(edge_index, np.int32)
    pl, ncs = _get_compiled(edge_index)
    layer_params = [(W1, a_src1, a_dst1, b1), (W2, a_src2, a_dst2, b2),
                    (W3, a_src3, a_dst3, None)]
    if FUSED:
        in_maps = fused_inputs(pl, x, layer_params)
        res = bass_utils.run_bass_kernel_spmd(ncs[0], in_maps,
                                              core_ids=list(range(8)))
        tot = np.sum([res.results[c]["y"] for c in range(8)], axis=0)
    else:
        hins = x_slices(pl, x)
        for l in range(3):
            W, a_s, a_d, b = layer_params[l]
            in_maps = layer_inputs(pl, l, hins, W, a_s, a_d, b)
            res = bass_utils.run_bass_kernel_spmd(ncs[l], in_maps,
                                                  core_ids=list(range(8)))
            if l < 2:
                hins = [res.results[c]["hout"] for c in range(8)]
            else:
                tot = np.sum([res.results[c]["y"] for c in range(8)], axis=0)
    return (tot / np.float32(N_NODES)
            + np.asarray(b3, np.float32)[None, :]).astype(np.float32)
